# revision 41
# baseline (speedup 1.0000x reference)
"""Trainium2 Bass kernel for nn_ActorCritic (gnn_message_passing).

Strategy: shard the BATCH axis (64 -> 8 per core) across the 8 NeuronCores;
softmax over actions is per-batch-row, so no collectives are needed.

Per core (8 batches as 4 pairs stacked on 128 partitions):
  1. Project first: per-pair tables T1' = task_embed @ W1_task written to a
     DRAM scratch as f16 256B rows (token = task id; both batches' 64
     projected dims per row).
  2. One HBM dma_gather per pair pulls the 8192 action rows (actions sorted
     by usv id on the host; probs are unpermuted on the host afterwards).
  3. PE transposes the gathered blocks into [H-pair, A] f16 PSUM; the usv
     part + (global@W1_g + b1) bias is a per-partition scalar within each
     sorted usv run, applied by the same tensor_scalar op that evacuates
     the transpose PSUM.
  4. ELU via min(exp(x),1) + max(x,0); the -1 folds into the next layer's
     bias (b2' = b2 - W2.sum(0)); b3 drops out of the softmax; the C/R
     halves are joined by accumulating two matmuls into the same PSUM.
  5. Weight-stationary f16 matmuls (blockdiag over the pair) for layers 2/3,
     softmax without max-subtraction, critic MLP in f32 on the side.
"""
import sys

if '/opt/trn_rl_repo' not in sys.path:
    sys.path.insert(0, '/opt/trn_rl_repo')

import numpy as np

B, NT, NU, H, A = 64, 1024, 32, 64, 8192
NCORES = 8
BPC = B // NCORES          # batches per core = 8
NPAIR = BPC // 2           # 4
CH = 512                   # action chunk (one PSUM bank)
SPAN = 2048                # elementwise span for SBUF passes
NSPAN = A // SPAN          # 4
CPS = SPAN // CH           # chunks per span = 4
NCHUNK = A // CH           # 16

_CACHE = {}


def _wrap_idx(ids: np.ndarray) -> np.ndarray:
    """gather index layout: idx j lives at [j % 16, j // 16], replicated
    across the 8 groups of 16 partitions."""
    a = ids.shape[0]
    w16 = np.zeros((16, a // 16), np.int16)
    w16[np.arange(a) % 16, np.arange(a) // 16] = ids.astype(np.int16)
    return np.tile(w16, (8, 1))


def _blockdiag(m: np.ndarray, n: int) -> np.ndarray:
    k, j = m.shape
    out = np.zeros((k * n, j * n), m.dtype)
    for i in range(n):
        out[i * k:(i + 1) * k, i * j:(i + 1) * j] = m
    return out


def _segments(sorted_usv: np.ndarray):
    """Per 512-chunk list of (rel_lo, rel_hi, u) for the sorted usv runs."""
    bounds = np.searchsorted(sorted_usv, np.arange(NU + 1))
    segs = [[] for _ in range(NCHUNK)]
    for u in range(NU):
        lo, hi = int(bounds[u]), int(bounds[u + 1])
        if hi <= lo:
            continue
        k0, k1 = lo // CH, (hi - 1) // CH
        for k in range(k0, k1 + 1):
            a0, a1 = max(lo, CH * k), min(hi, CH * (k + 1))
            segs[k].append((a0 - CH * k, a1 - CH * k, u))
    return segs


def _build_graph(segs):
    import concourse.bass as bass
    import concourse.mybir as mybir
    from concourse import bacc
    from concourse.tile import TileContext

    f32 = mybir.dt.float32
    f16 = mybir.dt.float16
    i16 = mybir.dt.int16
    AF = mybir.ActivationFunctionType
    OP = mybir.AluOpType

    nc = bacc.Bacc()

    # ---- dram parameters -------------------------------------------------
    task_e = nc.declare_dram_parameter("task_e", [BPC, NT, H], f32, isOutput=False)
    usv_e = nc.declare_dram_parameter("usv_e", [BPC, NU, H], f32, isOutput=False)
    glob = nc.declare_dram_parameter("glob", [BPC, H], f32, isOutput=False)
    idx_t = nc.declare_dram_parameter("idx_t", [128, A // 16], i16, isOutput=False)
    w1bd16 = nc.declare_dram_parameter("w1bd16", [128, 128], f16, isOutput=False)
    w1ubd16 = nc.declare_dram_parameter("w1ubd16", [128, 128], f16, isOutput=False)
    w1gbd = nc.declare_dram_parameter("w1gbd", [128, 128], f32, isOutput=False)
    b1_2 = nc.declare_dram_parameter("b1_2", [128, 1], f32, isOutput=False)
    w2bd16 = nc.declare_dram_parameter("w2bd16", [128, 64], f16, isOutput=False)
    b2q = nc.declare_dram_parameter("b2q", [128, 1], f32, isOutput=False)
    w3bd416 = nc.declare_dram_parameter("w3bd416", [128, 4], f16, isOutput=False)
    wc1 = nc.declare_dram_parameter("wc1", [H, 64], f32, isOutput=False)
    bc1c = nc.declare_dram_parameter("bc1c", [64, 1], f32, isOutput=False)
    wc2 = nc.declare_dram_parameter("wc2", [64, 32], f32, isOutput=False)
    bc2q = nc.declare_dram_parameter("bc2q", [32, 1], f32, isOutput=False)
    wc3 = nc.declare_dram_parameter("wc3", [32, 1], f32, isOutput=False)
    bc3q = nc.declare_dram_parameter("bc3q", [1, 1], f32, isOutput=False)
    ident = nc.declare_dram_parameter("ident", [128, 128], f32, isOutput=False)

    tbl_dram = nc.dram_tensor("tbl_dram", [NT, 512], f16)
    probs_out = nc.declare_dram_parameter("probs_out", [BPC, A], f16, isOutput=True)
    sv_out = nc.declare_dram_parameter("sv_out", [1, BPC], f32, isOutput=True)

    with TileContext(nc) as tc:
        with tc.tile_pool(name="const", bufs=1) as cst, \
             tc.tile_pool(name="pair", bufs=2) as pr, \
             tc.tile_pool(name="tables", bufs=3) as tbl, \
             tc.tile_pool(name="gath", bufs=1) as gpool, \
             tc.tile_pool(name="big", bufs=2) as big, \
             tc.tile_pool(name="spans", bufs=2) as sp, \
             tc.tile_pool(name="l2", bufs=2) as l2, \
             tc.tile_pool(name="out", bufs=1) as outp, \
             tc.tile_pool(name="ps_a", bufs=2, space="PSUM") as psa, \
             tc.tile_pool(name="ps_tr", bufs=2, space="PSUM") as pstr, \
             tc.tile_pool(name="ps_h2", bufs=2, space="PSUM") as ps2, \
             tc.tile_pool(name="ps_s", bufs=2, space="PSUM") as pss:

            def load_const(ext, shape, dtype=f32):
                t = cst.tile(shape, dtype, tag=ext.name)
                nc.sync.dma_start(out=t[:], in_=ext[:])
                return t

            identt = load_const(ident, [128, 128])
            identt16 = cst.tile([128, 128], f16, tag="ident16")
            nc.vector.tensor_copy(identt16[:], identt[:])
            idxt_sb = load_const(idx_t, [128, A // 16], i16)
            w1bd16_sb = load_const(w1bd16, [128, 128], f16)
            w1ubd16_sb = load_const(w1ubd16, [128, 128], f16)
            w1gbd_sb = load_const(w1gbd, [128, 128])
            b1_2_sb = load_const(b1_2, [128, 1])
            w2bd16_sb = load_const(w2bd16, [128, 64], f16)
            b2q_sb = load_const(b2q, [128, 1])
            w3bd416_sb = load_const(w3bd416, [128, 4], f16)
            wc1_sb = load_const(wc1, [H, 64])
            bc1c_sb = load_const(bc1c, [64, 1])
            wc2_sb = load_const(wc2, [64, 32])
            bc2q_sb = load_const(bc2q, [32, 1])
            wc3_sb = load_const(wc3, [32, 1])
            bc3q_sb = load_const(bc3q, [1, 1])

            # ---- globals, g1b1, critic (tiny, f32) ------------------------
            g2_sb = cst.tile([BPC, 128], f32)
            nc.sync.dma_start(out=g2_sb[:, 0:H], in_=glob[:])
            nc.sync.dma_start(out=g2_sb[:, H:128], in_=glob[:])
            ps_gT = psa.tile([128, BPC], f32, tag="A")
            nc.tensor.transpose(out=ps_gT[:], in_=g2_sb[:], identity=identt[:BPC, :BPC])
            gT2_sb = cst.tile([128, BPC], f32)
            nc.scalar.activation(gT2_sb[:], ps_gT[:], AF.Identity, bias=0.0, scale=1.0)

            ps_g1 = psa.tile([128, BPC], f32, tag="A")
            nc.tensor.matmul(ps_g1[:], w1gbd_sb[:], gT2_sb[:], start=True, stop=True)
            g1b1_sb = cst.tile([128, BPC], f32)
            nc.scalar.activation(g1b1_sb[:], ps_g1[:], AF.Identity, bias=b1_2_sb[:],
                                 scale=1.0)

            # critic
            ps_h1c = psa.tile([64, BPC], f32, tag="A")
            nc.tensor.matmul(ps_h1c[:], wc1_sb[:], gT2_sb[0:H, :], start=True, stop=True)
            ec = cst.tile([64, BPC], f32, tag="ec")
            rc = cst.tile([64, BPC], f32, tag="rc")
            nc.scalar.activation(ec[:], ps_h1c[:], AF.Exp, bias=bc1c_sb[:], scale=1.0)
            nc.scalar.activation(rc[:], ps_h1c[:], AF.Relu, bias=bc1c_sb[:], scale=1.0)
            nc.vector.tensor_scalar(ec[:], ec[:], 1.0, None, OP.min)
            h1ce = cst.tile([64, BPC], f32, tag="h1ce")
            nc.vector.tensor_tensor(h1ce[:], ec[:], rc[:], OP.add)
            ps_h2c = psa.tile([32, BPC], f32, tag="A")
            nc.tensor.matmul(ps_h2c[:], wc2_sb[:], h1ce[:], start=True, stop=True)
            ec2 = cst.tile([32, BPC], f32, tag="ec2")
            rc2 = cst.tile([32, BPC], f32, tag="rc2")
            nc.scalar.activation(ec2[:], ps_h2c[:], AF.Exp, bias=bc2q_sb[:], scale=1.0)
            nc.scalar.activation(rc2[:], ps_h2c[:], AF.Relu, bias=bc2q_sb[:], scale=1.0)
            nc.vector.tensor_scalar(ec2[:], ec2[:], 1.0, None, OP.min)
            h2ce = cst.tile([32, BPC], f32, tag="h2ce")
            nc.vector.tensor_tensor(h2ce[:], ec2[:], rc2[:], OP.add)
            ps_sv = psa.tile([1, BPC], f32, tag="A")
            nc.tensor.matmul(ps_sv[:], wc3_sb[:], h2ce[:], start=True, stop=True)
            sv_sb = cst.tile([1, BPC], f32, tag="svsb")
            nc.scalar.activation(sv_sb[:], ps_sv[:], AF.Identity, bias=bc3q_sb[:],
                                 scale=1.0)
            nc.sync.dma_start(out=sv_out[:], in_=sv_sb[:])

            # ---- preamble: one table for all pairs + usvcols + gathers -----
            table = tbl.tile([128, 8, 4, 128], f16, tag="table", bufs=1)
            usvcols = {}
            if True:
                for p in range(NPAIR):
                    b0, b1i = 2 * p, 2 * p + 1
                    dmae = (nc.sync, nc.scalar, nc.sync, nc.scalar)[p]
                    taskc = pr.tile([128, 2, 8, H], f32, tag="taskc", bufs=3)
                    for i, b in enumerate((b0, b1i)):
                        dmae.dma_start(
                            out=taskc[:, i, :, :],
                            in_=task_e[b].rearrange("(p r) h -> p r h", p=128))
                    taskc16 = pr.tile([128, 8, H], f16, tag="taskc16")
                    nc.vector.tensor_copy(taskc16[:], taskc[:, 1, :, :])
                    for half in range(2):
                        ps_taskT = pstr.tile([128, 512], f32, tag="tr")
                        for c in range(4):
                            cc = half * 4 + c
                            nc.tensor.transpose(
                                out=ps_taskT[0:H, 128 * c:128 * (c + 1)],
                                in_=taskc[:, 0, cc, :], identity=identt[:])
                            nc.tensor.matmul(
                                ps_taskT[H:128, 128 * c:128 * (c + 1)],
                                taskc16[:, cc, :], identt16[:],
                                start=True, stop=True, tile_position=(0, H))
                        taskT16 = pr.tile([128, 512], f16, tag="taskT16", bufs=2)
                        if half == 0:
                            nc.vector.tensor_copy(taskT16[:], ps_taskT[:])
                        else:
                            nc.scalar.activation(taskT16[:], ps_taskT[:],
                                                 AF.Identity, bias=0.0, scale=1.0)
                        for c in range(4):
                            s = half * 4 + c
                            ps_t1 = psa.tile([128, 128], f32, tag="A")
                            nc.tensor.matmul(ps_t1[:],
                                             taskT16[:, 128 * c:128 * (c + 1)],
                                             w1bd16_sb[:], start=True, stop=True)
                            if c % 2 == 0:
                                nc.vector.tensor_copy(table[:, s, p, :], ps_t1[:])
                            else:
                                nc.scalar.activation(table[:, s, p, :], ps_t1[:],
                                                     AF.Identity, bias=0.0,
                                                     scale=1.0)

                    # usvcol [128, 32] f32: col u = U1_pair[:, u] + g1 + b1
                    usvc = pr.tile([NU, 2, H], f32, tag="usvc", bufs=2)
                    for i, b in enumerate((b0, b1i)):
                        dmae.dma_start(out=usvc[:, i, :], in_=usv_e[b])
                    ps_usvT = psa.tile([128, NU], f32, tag="A")
                    nc.tensor.transpose(
                        out=ps_usvT[:],
                        in_=usvc[:].rearrange("u i h -> u (i h)"),
                        identity=identt[:NU, :NU])
                    u_sb16 = pr.tile([128, NU], f16, tag="usvT16")
                    nc.vector.tensor_copy(u_sb16[:], ps_usvT[:])
                    ps_u1 = psa.tile([128, NU], f32, tag="A")
                    nc.tensor.matmul(ps_u1[:], w1ubd16_sb[:], u_sb16[:],
                                     start=True, stop=True)
                    bias1 = pr.tile([128, 1], f32, tag="bias1")
                    nc.scalar.activation(bias1[0:H, :], g1b1_sb[0:H, b0:b0 + 1],
                                         AF.Identity, bias=0.0, scale=1.0)
                    nc.scalar.activation(bias1[H:128, :],
                                         g1b1_sb[H:128, b1i:b1i + 1],
                                         AF.Identity, bias=0.0, scale=1.0)
                    usvcol = tbl.tile([128, NU], f32, tag=f"usvcol{p}",
                                      name=f"usvcol_{p}")
                    nc.scalar.activation(usvcol[:], ps_u1[:], AF.Identity,
                                         bias=bias1[:], scale=1.0)
                    usvcols[p] = usvcol

                nc.sync.dma_start(
                    out=tbl_dram[:].rearrange("(p s) e -> p s e", p=128),
                    in_=table[:].rearrange("p s q e -> p s (q e)"))
                gath = gpool.tile([128, 64, 512], f16, tag="gath")
                for q in range(8):
                    nc.gpsimd.dma_gather(
                        out_ap=gath[:, 8 * q:8 * (q + 1), :],
                        in_ap=tbl_dram[:],
                        idxs_ap=idxt_sb[:, 64 * q:64 * (q + 1)],
                        num_idxs=1024,
                        num_idxs_reg=1024,
                        elem_size=512,
                        transpose=False,
                        queue_num=0,
                    )

            # ---- score targets --------------------------------------------
            es_g = [outp.tile([4, A], f16, tag=f"es{g}", name=f"es_g{g}")
                    for g in range(2)]
            sums_g = [outp.tile([4, NCHUNK], f32, tag=f"sums{g}", name=f"sums_g{g}")
                      for g in range(2)]

            # ---- main pipeline (2 pairs per group) -------------------------
            for grp in range(NPAIR // 2):
                subs = (2 * grp, 2 * grp + 1)
                for span_i in range(NSPAN):
                    cr = {}
                    for si in range(2):
                        s_pair = 2 * grp + si
                        usvcol = usvcols[s_pair]
                        hp = big.tile([128, SPAN], f16, tag=f"h1p{si}")
                        for c4 in range(CPS):
                            k = span_i * CPS + c4
                            ps_tr = pstr.tile([128, CH], f16, tag="tr")
                            for b in range(CH // 128):
                                blk = (CH // 128) * k + b
                                nc.tensor.transpose(
                                    out=ps_tr[:, 128 * b:128 * (b + 1)],
                                    in_=gath[:, blk,
                                             128 * s_pair:128 * (s_pair + 1)],
                                    identity=identt16[:])
                            for (a0, a1, u) in segs[k]:
                                nc.vector.tensor_scalar(
                                    hp[:, CH * c4 + a0:CH * c4 + a1],
                                    ps_tr[:, a0:a1],
                                    usvcol[:, u:u + 1], None, OP.add)
                        et0 = sp.tile([128, SPAN], f16, tag="e0")
                        nc.scalar.activation(et0[:], hp[:], AF.Exp, bias=0.0, scale=1.0)
                        et = sp.tile([128, SPAN], f16, tag=f"e1_{si}")
                        nc.vector.tensor_scalar(et[:], et0[:], 1.0, None, OP.min)
                        rt = sp.tile([128, SPAN], f16, tag=f"r1_{si}")
                        nc.vector.tensor_scalar(rt[:], hp[:], 0.0, None, OP.max)
                        cr[si] = (et, rt)
                    for c4 in range(CPS):
                        k = span_i * CPS + c4
                        off = CH * c4
                        ps_h2 = ps2.tile([128, CH], f32, tag="h2")
                        for si in range(2):
                            et, rt = cr[si]
                            nc.tensor.matmul(
                                ps_h2[64 * si:64 * si + 64, :],
                                w2bd16_sb[:], et[:, off:off + CH],
                                start=True, stop=False, tile_position=(0, 64 * si))
                            nc.tensor.matmul(
                                ps_h2[64 * si:64 * si + 64, :],
                                w2bd16_sb[:], rt[:, off:off + CH],
                                start=False, stop=True, tile_position=(0, 64 * si))
                        e2t0 = l2.tile([128, CH], f16, tag="e20")
                        nc.scalar.activation(e2t0[:], ps_h2[:], AF.Exp,
                                             bias=b2q_sb[:], scale=1.0)
                        e2t = l2.tile([128, CH], f16, tag="e2")
                        nc.vector.tensor_scalar(e2t[:], e2t0[:], 1.0, None, OP.min)
                        r2t = l2.tile([128, CH], f16, tag="r2")
                        nc.vector.tensor_scalar(r2t[:], ps_h2[:], b2q_sb[:], 0.0,
                                                OP.add, OP.max)
                        ps_sk = pss.tile([4, CH], f32, tag="s")
                        nc.tensor.matmul(ps_sk[:], w3bd416_sb[:], e2t[:],
                                         start=True, stop=False)
                        nc.tensor.matmul(ps_sk[:], w3bd416_sb[:], r2t[:],
                                         start=False, stop=True)
                        nc.scalar.activation(
                            es_g[grp][:, CH * k:CH * (k + 1)], ps_sk[:],
                            AF.Exp, bias=0.0, scale=1.0,
                            accum_out=sums_g[grp][:, k:k + 1])

            # ---- softmax normalisation ------------------------------------
            es8 = outp.tile([BPC, A], f16, tag="es8")
            sums8 = outp.tile([BPC, NCHUNK], f32, tag="sums8")
            for g in range(2):
                nc.sync.dma_start(out=es8[4 * g:4 * g + 4, :], in_=es_g[g][:])
                nc.sync.dma_start(out=sums8[4 * g:4 * g + 4, :], in_=sums_g[g][:])
            ssum = outp.tile([BPC, 1], f32, tag="ssum")
            nc.vector.tensor_reduce(ssum[:], sums8[:], mybir.AxisListType.X, OP.add)
            rsum = outp.tile([BPC, 1], f32, tag="rsum")
            nc.vector.reciprocal(rsum[:], ssum[:])
            nc.vector.tensor_scalar(es8[:], es8[:], rsum[:], None, OP.mult)
            nc.sync.dma_start(out=probs_out[:], in_=es8[:])

    nc.compile()
    return nc


def _prep_static(inputs):
    """Host-side marshalling of weights/indices (tiny, O(weights + A))."""
    f = lambda x: np.asarray(x, np.float32)
    W1, b1 = f(inputs["W1"]), f(inputs["b1"])
    W2, b2 = f(inputs["W2"]), f(inputs["b2"])
    W3 = f(inputs["W3"])
    Wc1, bc1 = f(inputs["Wc1"]), f(inputs["bc1"])
    Wc2, bc2 = f(inputs["Wc2"]), f(inputs["bc2"])
    Wc3, bc3 = f(inputs["Wc3"]), f(inputs["bc3"])
    W1_t, W1_u, W1_g = W1[0:H], W1[H:2 * H], W1[2 * H:3 * H]

    task_ids = np.asarray(inputs["task_ids"])
    usv_ids = np.asarray(inputs["usv_ids"])
    order = np.argsort(usv_ids, kind="stable")

    d = {
        "idx_t": _wrap_idx(task_ids[order]),
        "w1bd16": _blockdiag(W1_t, 2).astype(np.float16),
        "w1ubd16": _blockdiag(W1_u, 2).astype(np.float16),
        "w1gbd": _blockdiag(W1_g, 2),
        "b1_2": np.tile(b1, 2)[:, None],
        "w2bd16": _blockdiag(W2, 2).astype(np.float16),
        "b2q": np.tile(b2 - W2.sum(0), 4)[:, None],
        "w3bd416": _blockdiag(W3, 4).astype(np.float16),
        "wc1": Wc1,
        "bc1c": bc1[:, None],
        "wc2": Wc2,
        "bc2q": (bc2 - Wc2.sum(0))[:, None],
        "wc3": Wc3,
        "bc3q": (bc3 - Wc3.sum(0)).reshape(1, 1),
        "ident": np.eye(128, dtype=np.float32),
    }
    return {k: np.ascontiguousarray(v) for k, v in d.items()}, order


def kernel(**inputs):
    from concourse.bass_utils import run_bass_kernel_spmd

    task_ids = np.asarray(inputs["task_ids"])
    usv_ids = np.asarray(inputs["usv_ids"])
    key = (task_ids.tobytes(), usv_ids.tobytes())
    if _CACHE.get("key") != key:
        order = np.argsort(usv_ids, kind="stable")
        _CACHE["nc"] = _build_graph(_segments(usv_ids[order]))
        _CACHE["key"] = key
    nc = _CACHE["nc"]

    static, order = _prep_static(inputs)
    inv = np.empty(A, np.int64)
    inv[order] = np.arange(A)
    task = np.ascontiguousarray(np.asarray(inputs["task_embed"], np.float32))
    usv = np.ascontiguousarray(np.asarray(inputs["usv_embed"], np.float32))
    glob = np.ascontiguousarray(np.asarray(inputs["global_embed"], np.float32))

    in_maps = []
    for c in range(NCORES):
        sl = slice(c * BPC, (c + 1) * BPC)
        m = dict(static)
        m["task_e"] = task[sl]
        m["usv_e"] = usv[sl]
        m["glob"] = glob[sl]
        in_maps.append(m)

    res = None
    for attempt in range(3):
        try:
            res = run_bass_kernel_spmd(nc, in_maps, core_ids=list(range(NCORES)))
            break
        except Exception:
            if attempt == 2:
                raise
    outs = res.results
    probs = np.concatenate([outs[c]["probs_out"] for c in range(NCORES)], axis=0)
    probs = probs[:, inv]
    sv = np.concatenate([outs[c]["sv_out"][0] for c in range(NCORES)], axis=0)[:, None]
    return probs.astype(np.float32), sv.astype(np.float32)


# revision 43
# speedup vs baseline: 1.0052x; 1.0052x over previous
"""Trainium2 Bass kernel for nn_ActorCritic (gnn_message_passing).

Measured: ~241 us HW exec (neuron-profile, 8 NeuronCores), rel err ~2.7e-3.

Strategy: shard the BATCH axis (64 -> 8 per core) across the 8 NeuronCores;
softmax over actions is per-batch-row, so no collectives are needed at all
(vs. the action-axis hint, which would need an allgather and 8x the HBM
traffic for task_embed).

Per core (8 batches as 4 pairs stacked on 128 partitions):
  1. Project first: T1' = task_embed @ W1_task per pair (f16 matmuls via
     PE-transposed task blocks), written to one DRAM scratch table with
     1 KB rows: row t = all 8 batches' 64 projected dims for task t.
  2. Actions are sorted by usv id on the host (indices are inputs, so the
     graph is JIT-specialized; probs are unpermuted on the host).  8
     dma_gather calls (1024 descriptors each -- the SWDGE ring cap) fetch
     the 8192 action rows; PE is_transpose flips each [128,128] f16 block
     into [H-pair, actions] f16 PSUM.
  3. The usv part + (global@W1_g + b1) bias is a per-partition scalar
     within each sorted usv run, so a single one-input tensor_scalar per
     run segment evacuates the transpose PSUM and applies it.
  4. ELU via min(exp(x),1) + max(x,0); the -1 folds into the next layer's
     bias (b2' = b2 - W2.sum(0)); b3 drops out of the softmax (shift
     invariance); the C/R halves are joined by accumulating two matmuls
     into the same PSUM (no join pass).
  5. Weight-stationary f16 matmuls (blockdiag over the batch pair) for
     layers 2/3; the mm3-PSUM evacuation doubles as the softmax exp with
     accumulated partial sums; softmax runs without max-subtraction
     (scores are O(+-5)); critic MLP in f32 on the side; f16 output is
     cast to f32 on the host.
"""
import sys

if '/opt/trn_rl_repo' not in sys.path:
    sys.path.insert(0, '/opt/trn_rl_repo')

import numpy as np

B, NT, NU, H, A = 64, 1024, 32, 64, 8192
NCORES = 8
BPC = B // NCORES          # batches per core = 8
NPAIR = BPC // 2           # 4
CH = 512                   # action chunk (one PSUM bank)
SPAN = 2048                # elementwise span for SBUF passes
NSPAN = A // SPAN          # 4
CPS = SPAN // CH           # chunks per span = 4
NCHUNK = A // CH           # 16

_CACHE = {}


def _wrap_idx(ids: np.ndarray) -> np.ndarray:
    """gather index layout: idx j lives at [j % 16, j // 16], replicated
    across the 8 groups of 16 partitions."""
    a = ids.shape[0]
    w16 = np.zeros((16, a // 16), np.int16)
    w16[np.arange(a) % 16, np.arange(a) // 16] = ids.astype(np.int16)
    return np.tile(w16, (8, 1))


def _blockdiag(m: np.ndarray, n: int) -> np.ndarray:
    k, j = m.shape
    out = np.zeros((k * n, j * n), m.dtype)
    for i in range(n):
        out[i * k:(i + 1) * k, i * j:(i + 1) * j] = m
    return out


def _segments(sorted_usv: np.ndarray):
    """Per 512-chunk list of (rel_lo, rel_hi, u) for the sorted usv runs."""
    bounds = np.searchsorted(sorted_usv, np.arange(NU + 1))
    segs = [[] for _ in range(NCHUNK)]
    for u in range(NU):
        lo, hi = int(bounds[u]), int(bounds[u + 1])
        if hi <= lo:
            continue
        k0, k1 = lo // CH, (hi - 1) // CH
        for k in range(k0, k1 + 1):
            a0, a1 = max(lo, CH * k), min(hi, CH * (k + 1))
            segs[k].append((a0 - CH * k, a1 - CH * k, u))
    return segs


def _build_graph(segs):
    import concourse.bass as bass
    import concourse.mybir as mybir
    from concourse import bacc
    from concourse.tile import TileContext

    f32 = mybir.dt.float32
    f16 = mybir.dt.float16
    i16 = mybir.dt.int16
    AF = mybir.ActivationFunctionType
    OP = mybir.AluOpType

    nc = bacc.Bacc()

    # ---- dram parameters -------------------------------------------------
    task_e = nc.declare_dram_parameter("task_e", [BPC, NT, H], f32, isOutput=False)
    usv_e = nc.declare_dram_parameter("usv_e", [BPC, NU, H], f32, isOutput=False)
    glob = nc.declare_dram_parameter("glob", [BPC, H], f32, isOutput=False)
    idx_t = nc.declare_dram_parameter("idx_t", [128, A // 16], i16, isOutput=False)
    w1bd16 = nc.declare_dram_parameter("w1bd16", [128, 128], f16, isOutput=False)
    w1ubd16 = nc.declare_dram_parameter("w1ubd16", [128, 128], f16, isOutput=False)
    w1gbd = nc.declare_dram_parameter("w1gbd", [128, 128], f32, isOutput=False)
    b1_2 = nc.declare_dram_parameter("b1_2", [128, 1], f32, isOutput=False)
    w2bd16 = nc.declare_dram_parameter("w2bd16", [128, 64], f16, isOutput=False)
    b2q = nc.declare_dram_parameter("b2q", [128, 1], f32, isOutput=False)
    w3bd416 = nc.declare_dram_parameter("w3bd416", [128, 4], f16, isOutput=False)
    wc1 = nc.declare_dram_parameter("wc1", [H, 64], f32, isOutput=False)
    bc1c = nc.declare_dram_parameter("bc1c", [64, 1], f32, isOutput=False)
    wc2 = nc.declare_dram_parameter("wc2", [64, 32], f32, isOutput=False)
    bc2q = nc.declare_dram_parameter("bc2q", [32, 1], f32, isOutput=False)
    wc3 = nc.declare_dram_parameter("wc3", [32, 1], f32, isOutput=False)
    bc3q = nc.declare_dram_parameter("bc3q", [1, 1], f32, isOutput=False)
    ident = nc.declare_dram_parameter("ident", [128, 128], f32, isOutput=False)

    tbl_dram = nc.dram_tensor("tbl_dram", [NT, 512], f16)
    probs_out = nc.declare_dram_parameter("probs_out", [BPC, A], f16, isOutput=True)
    sv_out = nc.declare_dram_parameter("sv_out", [1, BPC], f32, isOutput=True)

    with TileContext(nc) as tc:
        with tc.tile_pool(name="const", bufs=1) as cst, \
             tc.tile_pool(name="pair", bufs=2) as pr, \
             tc.tile_pool(name="tables", bufs=3) as tbl, \
             tc.tile_pool(name="gath", bufs=1) as gpool, \
             tc.tile_pool(name="big", bufs=2) as big, \
             tc.tile_pool(name="spans", bufs=2) as sp, \
             tc.tile_pool(name="l2", bufs=2) as l2, \
             tc.tile_pool(name="out", bufs=1) as outp, \
             tc.tile_pool(name="ps_a", bufs=2, space="PSUM") as psa, \
             tc.tile_pool(name="ps_tr", bufs=2, space="PSUM") as pstr, \
             tc.tile_pool(name="ps_h2", bufs=2, space="PSUM") as ps2, \
             tc.tile_pool(name="ps_s", bufs=2, space="PSUM") as pss:

            def load_const(ext, shape, dtype=f32):
                t = cst.tile(shape, dtype, tag=ext.name)
                nc.sync.dma_start(out=t[:], in_=ext[:])
                return t

            identt = load_const(ident, [128, 128])
            identt16 = cst.tile([128, 128], f16, tag="ident16")
            nc.vector.tensor_copy(identt16[:], identt[:])
            idxt_sb = load_const(idx_t, [128, A // 16], i16)
            w1bd16_sb = load_const(w1bd16, [128, 128], f16)
            w1ubd16_sb = load_const(w1ubd16, [128, 128], f16)
            w1gbd_sb = load_const(w1gbd, [128, 128])
            b1_2_sb = load_const(b1_2, [128, 1])
            w2bd16_sb = load_const(w2bd16, [128, 64], f16)
            b2q_sb = load_const(b2q, [128, 1])
            w3bd416_sb = load_const(w3bd416, [128, 4], f16)
            wc1_sb = load_const(wc1, [H, 64])
            bc1c_sb = load_const(bc1c, [64, 1])
            wc2_sb = load_const(wc2, [64, 32])
            bc2q_sb = load_const(bc2q, [32, 1])
            wc3_sb = load_const(wc3, [32, 1])
            bc3q_sb = load_const(bc3q, [1, 1])

            # ---- globals, g1b1, critic (tiny, f32) ------------------------
            g2_sb = cst.tile([BPC, 128], f32)
            nc.sync.dma_start(out=g2_sb[:, 0:H], in_=glob[:])
            nc.sync.dma_start(out=g2_sb[:, H:128], in_=glob[:])
            ps_gT = psa.tile([128, BPC], f32, tag="A")
            nc.tensor.transpose(out=ps_gT[:], in_=g2_sb[:], identity=identt[:BPC, :BPC])
            gT2_sb = cst.tile([128, BPC], f32)
            nc.scalar.activation(gT2_sb[:], ps_gT[:], AF.Identity, bias=0.0, scale=1.0)

            ps_g1 = psa.tile([128, BPC], f32, tag="A")
            nc.tensor.matmul(ps_g1[:], w1gbd_sb[:], gT2_sb[:], start=True, stop=True)
            g1b1_sb = cst.tile([128, BPC], f32)
            nc.scalar.activation(g1b1_sb[:], ps_g1[:], AF.Identity, bias=b1_2_sb[:],
                                 scale=1.0)

            # critic
            ps_h1c = psa.tile([64, BPC], f32, tag="A")
            nc.tensor.matmul(ps_h1c[:], wc1_sb[:], gT2_sb[0:H, :], start=True, stop=True)
            ec = cst.tile([64, BPC], f32, tag="ec")
            rc = cst.tile([64, BPC], f32, tag="rc")
            nc.scalar.activation(ec[:], ps_h1c[:], AF.Exp, bias=bc1c_sb[:], scale=1.0)
            nc.scalar.activation(rc[:], ps_h1c[:], AF.Relu, bias=bc1c_sb[:], scale=1.0)
            nc.vector.tensor_scalar(ec[:], ec[:], 1.0, None, OP.min)
            h1ce = cst.tile([64, BPC], f32, tag="h1ce")
            nc.vector.tensor_tensor(h1ce[:], ec[:], rc[:], OP.add)
            ps_h2c = psa.tile([32, BPC], f32, tag="A")
            nc.tensor.matmul(ps_h2c[:], wc2_sb[:], h1ce[:], start=True, stop=True)
            ec2 = cst.tile([32, BPC], f32, tag="ec2")
            rc2 = cst.tile([32, BPC], f32, tag="rc2")
            nc.scalar.activation(ec2[:], ps_h2c[:], AF.Exp, bias=bc2q_sb[:], scale=1.0)
            nc.scalar.activation(rc2[:], ps_h2c[:], AF.Relu, bias=bc2q_sb[:], scale=1.0)
            nc.vector.tensor_scalar(ec2[:], ec2[:], 1.0, None, OP.min)
            h2ce = cst.tile([32, BPC], f32, tag="h2ce")
            nc.vector.tensor_tensor(h2ce[:], ec2[:], rc2[:], OP.add)
            ps_sv = psa.tile([1, BPC], f32, tag="A")
            nc.tensor.matmul(ps_sv[:], wc3_sb[:], h2ce[:], start=True, stop=True)
            sv_sb = cst.tile([1, BPC], f32, tag="svsb")
            nc.scalar.activation(sv_sb[:], ps_sv[:], AF.Identity, bias=bc3q_sb[:],
                                 scale=1.0)
            nc.sync.dma_start(out=sv_out[:], in_=sv_sb[:])

            # ---- preamble: one table for all pairs + usvcols + gathers -----
            table = tbl.tile([128, 8, 4, 128], f16, tag="table", bufs=1)
            usvcols = {}
            if True:
                for p in range(NPAIR):
                    b0, b1i = 2 * p, 2 * p + 1
                    dmae = (nc.sync, nc.scalar, nc.sync, nc.scalar)[p]
                    taskc = pr.tile([128, 2, 8, H], f32, tag="taskc", bufs=3)
                    for i, b in enumerate((b0, b1i)):
                        dmae.dma_start(
                            out=taskc[:, i, :, :],
                            in_=task_e[b].rearrange("(p r) h -> p r h", p=128))
                    taskc16 = pr.tile([128, 8, H], f16, tag="taskc16")
                    nc.vector.tensor_copy(taskc16[:], taskc[:, 1, :, :])
                    for half in range(2):
                        ps_taskT = pstr.tile([128, 512], f32, tag="tr")
                        for c in range(4):
                            cc = half * 4 + c
                            nc.tensor.transpose(
                                out=ps_taskT[0:H, 128 * c:128 * (c + 1)],
                                in_=taskc[:, 0, cc, :], identity=identt[:])
                            nc.tensor.matmul(
                                ps_taskT[H:128, 128 * c:128 * (c + 1)],
                                taskc16[:, cc, :], identt16[:],
                                start=True, stop=True, tile_position=(0, H))
                        taskT16 = pr.tile([128, 512], f16, tag="taskT16", bufs=2)
                        if half == 0:
                            nc.vector.tensor_copy(taskT16[:], ps_taskT[:])
                        else:
                            nc.scalar.activation(taskT16[:], ps_taskT[:],
                                                 AF.Identity, bias=0.0, scale=1.0)
                        for c in range(4):
                            s = half * 4 + c
                            ps_t1 = psa.tile([128, 128], f32, tag="A")
                            nc.tensor.matmul(ps_t1[:],
                                             taskT16[:, 128 * c:128 * (c + 1)],
                                             w1bd16_sb[:], start=True, stop=True)
                            if c % 2 == 0:
                                nc.vector.tensor_copy(table[:, s, p, :], ps_t1[:])
                            else:
                                nc.scalar.activation(table[:, s, p, :], ps_t1[:],
                                                     AF.Identity, bias=0.0,
                                                     scale=1.0)

                    # usvcol [128, 32] f32: col u = U1_pair[:, u] + g1 + b1
                    usvc = pr.tile([NU, 2, H], f32, tag="usvc", bufs=2)
                    for i, b in enumerate((b0, b1i)):
                        dmae.dma_start(out=usvc[:, i, :], in_=usv_e[b])
                    ps_usvT = psa.tile([128, NU], f32, tag="A")
                    nc.tensor.transpose(
                        out=ps_usvT[:],
                        in_=usvc[:].rearrange("u i h -> u (i h)"),
                        identity=identt[:NU, :NU])
                    u_sb16 = pr.tile([128, NU], f16, tag="usvT16")
                    nc.vector.tensor_copy(u_sb16[:], ps_usvT[:])
                    ps_u1 = psa.tile([128, NU], f32, tag="A")
                    nc.tensor.matmul(ps_u1[:], w1ubd16_sb[:], u_sb16[:],
                                     start=True, stop=True)
                    bias1 = pr.tile([128, 1], f32, tag="bias1")
                    nc.scalar.activation(bias1[0:H, :], g1b1_sb[0:H, b0:b0 + 1],
                                         AF.Identity, bias=0.0, scale=1.0)
                    nc.scalar.activation(bias1[H:128, :],
                                         g1b1_sb[H:128, b1i:b1i + 1],
                                         AF.Identity, bias=0.0, scale=1.0)
                    usvcol = tbl.tile([128, NU], f32, tag=f"usvcol{p}",
                                      name=f"usvcol_{p}")
                    nc.scalar.activation(usvcol[:], ps_u1[:], AF.Identity,
                                         bias=bias1[:], scale=1.0)
                    usvcols[p] = usvcol

                nc.sync.dma_start(
                    out=tbl_dram[:].rearrange("(p s) e -> p s e", p=128),
                    in_=table[:].rearrange("p s q e -> p s (q e)"))
                gath = gpool.tile([128, 64, 512], f16, tag="gath")
                for q in range(8):
                    nc.gpsimd.dma_gather(
                        out_ap=gath[:, 8 * q:8 * (q + 1), :],
                        in_ap=tbl_dram[:],
                        idxs_ap=idxt_sb[:, 64 * q:64 * (q + 1)],
                        num_idxs=1024,
                        num_idxs_reg=1024,
                        elem_size=512,
                        transpose=False,
                        queue_num=0,
                    )

            # ---- score targets --------------------------------------------
            es_g = [outp.tile([4, A], f16, tag=f"es{g}", name=f"es_g{g}")
                    for g in range(2)]
            sums_g = [outp.tile([4, NCHUNK], f32, tag=f"sums{g}", name=f"sums_g{g}")
                      for g in range(2)]

            # ---- main pipeline (2 pairs per group) -------------------------
            for span_i in range(NSPAN):
                for grp in range(NPAIR // 2):
                    subs = (2 * grp, 2 * grp + 1)
                    cr = {}
                    for si in range(2):
                        s_pair = 2 * grp + si
                        usvcol = usvcols[s_pair]
                        hp = big.tile([128, SPAN], f16, tag=f"h1p{si}")
                        for c4 in range(CPS):
                            k = span_i * CPS + c4
                            ps_tr = pstr.tile([128, CH], f16, tag="tr")
                            for b in range(CH // 128):
                                blk = (CH // 128) * k + b
                                nc.tensor.transpose(
                                    out=ps_tr[:, 128 * b:128 * (b + 1)],
                                    in_=gath[:, blk,
                                             128 * s_pair:128 * (s_pair + 1)],
                                    identity=identt16[:])
                            for (a0, a1, u) in segs[k]:
                                nc.vector.tensor_scalar(
                                    hp[:, CH * c4 + a0:CH * c4 + a1],
                                    ps_tr[:, a0:a1],
                                    usvcol[:, u:u + 1], None, OP.add)
                        et0 = sp.tile([128, SPAN], f16, tag="e0")
                        nc.scalar.activation(et0[:], hp[:], AF.Exp, bias=0.0, scale=1.0)
                        et = sp.tile([128, SPAN], f16, tag=f"e1_{si}")
                        nc.vector.tensor_scalar(et[:], et0[:], 1.0, None, OP.min)
                        rt = sp.tile([128, SPAN], f16, tag=f"r1_{si}")
                        nc.vector.tensor_scalar(rt[:], hp[:], 0.0, None, OP.max)
                        cr[si] = (et, rt)
                    for c4 in range(CPS):
                        k = span_i * CPS + c4
                        off = CH * c4
                        ps_h2 = ps2.tile([128, CH], f32, tag="h2")
                        for si in range(2):
                            et, rt = cr[si]
                            nc.tensor.matmul(
                                ps_h2[64 * si:64 * si + 64, :],
                                w2bd16_sb[:], et[:, off:off + CH],
                                start=True, stop=False, tile_position=(0, 64 * si))
                            nc.tensor.matmul(
                                ps_h2[64 * si:64 * si + 64, :],
                                w2bd16_sb[:], rt[:, off:off + CH],
                                start=False, stop=True, tile_position=(0, 64 * si))
                        e2t0 = l2.tile([128, CH], f16, tag="e20")
                        nc.scalar.activation(e2t0[:], ps_h2[:], AF.Exp,
                                             bias=b2q_sb[:], scale=1.0)
                        e2t = l2.tile([128, CH], f16, tag="e2")
                        nc.vector.tensor_scalar(e2t[:], e2t0[:], 1.0, None, OP.min)
                        r2t = l2.tile([128, CH], f16, tag="r2")
                        nc.vector.tensor_scalar(r2t[:], ps_h2[:], b2q_sb[:], 0.0,
                                                OP.add, OP.max)
                        ps_sk = pss.tile([4, CH], f32, tag="s")
                        nc.tensor.matmul(ps_sk[:], w3bd416_sb[:], e2t[:],
                                         start=True, stop=False)
                        nc.tensor.matmul(ps_sk[:], w3bd416_sb[:], r2t[:],
                                         start=False, stop=True)
                        nc.scalar.activation(
                            es_g[grp][:, CH * k:CH * (k + 1)], ps_sk[:],
                            AF.Exp, bias=0.0, scale=1.0,
                            accum_out=sums_g[grp][:, k:k + 1])

            # ---- softmax normalisation ------------------------------------
            es8 = outp.tile([BPC, A], f16, tag="es8")
            sums8 = outp.tile([BPC, NCHUNK], f32, tag="sums8")
            for g in range(2):
                nc.sync.dma_start(out=es8[4 * g:4 * g + 4, :], in_=es_g[g][:])
                nc.sync.dma_start(out=sums8[4 * g:4 * g + 4, :], in_=sums_g[g][:])
            ssum = outp.tile([BPC, 1], f32, tag="ssum")
            nc.vector.tensor_reduce(ssum[:], sums8[:], mybir.AxisListType.X, OP.add)
            rsum = outp.tile([BPC, 1], f32, tag="rsum")
            nc.vector.reciprocal(rsum[:], ssum[:])
            nc.vector.tensor_scalar(es8[:], es8[:], rsum[:], None, OP.mult)
            nc.sync.dma_start(out=probs_out[:], in_=es8[:])

    nc.compile()
    return nc


def _prep_static(inputs):
    """Host-side marshalling of weights/indices (tiny, O(weights + A))."""
    f = lambda x: np.asarray(x, np.float32)
    W1, b1 = f(inputs["W1"]), f(inputs["b1"])
    W2, b2 = f(inputs["W2"]), f(inputs["b2"])
    W3 = f(inputs["W3"])
    Wc1, bc1 = f(inputs["Wc1"]), f(inputs["bc1"])
    Wc2, bc2 = f(inputs["Wc2"]), f(inputs["bc2"])
    Wc3, bc3 = f(inputs["Wc3"]), f(inputs["bc3"])
    W1_t, W1_u, W1_g = W1[0:H], W1[H:2 * H], W1[2 * H:3 * H]

    task_ids = np.asarray(inputs["task_ids"])
    usv_ids = np.asarray(inputs["usv_ids"])
    order = np.argsort(usv_ids, kind="stable")

    d = {
        "idx_t": _wrap_idx(task_ids[order]),
        "w1bd16": _blockdiag(W1_t, 2).astype(np.float16),
        "w1ubd16": _blockdiag(W1_u, 2).astype(np.float16),
        "w1gbd": _blockdiag(W1_g, 2),
        "b1_2": np.tile(b1, 2)[:, None],
        "w2bd16": _blockdiag(W2, 2).astype(np.float16),
        "b2q": np.tile(b2 - W2.sum(0), 4)[:, None],
        "w3bd416": _blockdiag(W3, 4).astype(np.float16),
        "wc1": Wc1,
        "bc1c": bc1[:, None],
        "wc2": Wc2,
        "bc2q": (bc2 - Wc2.sum(0))[:, None],
        "wc3": Wc3,
        "bc3q": (bc3 - Wc3.sum(0)).reshape(1, 1),
        "ident": np.eye(128, dtype=np.float32),
    }
    return {k: np.ascontiguousarray(v) for k, v in d.items()}, order


def kernel(**inputs):
    from concourse.bass_utils import run_bass_kernel_spmd

    task_ids = np.asarray(inputs["task_ids"])
    usv_ids = np.asarray(inputs["usv_ids"])
    key = (task_ids.tobytes(), usv_ids.tobytes())
    if _CACHE.get("key") != key:
        order = np.argsort(usv_ids, kind="stable")
        _CACHE["nc"] = _build_graph(_segments(usv_ids[order]))
        _CACHE["key"] = key
    nc = _CACHE["nc"]

    static, order = _prep_static(inputs)
    inv = np.empty(A, np.int64)
    inv[order] = np.arange(A)
    task = np.ascontiguousarray(np.asarray(inputs["task_embed"], np.float32))
    usv = np.ascontiguousarray(np.asarray(inputs["usv_embed"], np.float32))
    glob = np.ascontiguousarray(np.asarray(inputs["global_embed"], np.float32))

    in_maps = []
    for c in range(NCORES):
        sl = slice(c * BPC, (c + 1) * BPC)
        m = dict(static)
        m["task_e"] = task[sl]
        m["usv_e"] = usv[sl]
        m["glob"] = glob[sl]
        in_maps.append(m)

    res = None
    for attempt in range(3):
        try:
            res = run_bass_kernel_spmd(nc, in_maps, core_ids=list(range(NCORES)))
            break
        except Exception:
            if attempt == 2:
                raise
    outs = res.results
    probs = np.concatenate([outs[c]["probs_out"] for c in range(NCORES)], axis=0)
    probs = probs[:, inv]
    sv = np.concatenate([outs[c]["sv_out"][0] for c in range(NCORES)], axis=0)[:, None]
    return probs.astype(np.float32), sv.astype(np.float32)


# revision 44
# speedup vs baseline: 1.0054x; 1.0002x over previous
"""Trainium2 Bass kernel for nn_ActorCritic (gnn_message_passing).

Measured: ~241 us HW exec (neuron-profile, 8 NeuronCores), rel err ~2.7e-3.

Strategy: shard the BATCH axis (64 -> 8 per core) across the 8 NeuronCores;
softmax over actions is per-batch-row, so no collectives are needed at all
(vs. the action-axis hint, which would need an allgather and 8x the HBM
traffic for task_embed).

Per core (8 batches as 4 pairs stacked on 128 partitions):
  1. Project first: T1' = task_embed @ W1_task per pair (f16 matmuls via
     PE-transposed task blocks), written to one DRAM scratch table with
     1 KB rows: row t = all 8 batches' 64 projected dims for task t.
  2. Actions are sorted by usv id on the host (indices are inputs, so the
     graph is JIT-specialized; probs are unpermuted on the host).  8
     dma_gather calls (1024 descriptors each -- the SWDGE ring cap) fetch
     the 8192 action rows; PE is_transpose flips each [128,128] f16 block
     into [H-pair, actions] f16 PSUM.
  3. The usv part + (global@W1_g + b1) bias is a per-partition scalar
     within each sorted usv run, so a single one-input tensor_scalar per
     run segment evacuates the transpose PSUM and applies it.
  4. ELU via min(exp(x),1) + max(x,0); the -1 folds into the next layer's
     bias (b2' = b2 - W2.sum(0)); b3 drops out of the softmax (shift
     invariance); the C/R halves are joined by accumulating two matmuls
     into the same PSUM (no join pass).
  5. Weight-stationary f16 matmuls (blockdiag over the batch pair) for
     layers 2/3; the mm3-PSUM evacuation doubles as the softmax exp with
     accumulated partial sums; softmax runs without max-subtraction
     (scores are O(+-5)); critic MLP in f32 on the side; f16 output is
     cast to f32 on the host.
"""
import sys

if '/opt/trn_rl_repo' not in sys.path:
    sys.path.insert(0, '/opt/trn_rl_repo')

import numpy as np

B, NT, NU, H, A = 64, 1024, 32, 64, 8192
NCORES = 8
BPC = B // NCORES          # batches per core = 8
NPAIR = BPC // 2           # 4
CH = 512                   # action chunk (one PSUM bank)
SPAN = 2048                # elementwise span for SBUF passes
NSPAN = A // SPAN          # 4
CPS = SPAN // CH           # chunks per span = 4
NCHUNK = A // CH           # 16

_CACHE = {}


def _wrap_idx(ids: np.ndarray) -> np.ndarray:
    """gather index layout: idx j lives at [j % 16, j // 16], replicated
    across the 8 groups of 16 partitions."""
    a = ids.shape[0]
    w16 = np.zeros((16, a // 16), np.int16)
    w16[np.arange(a) % 16, np.arange(a) // 16] = ids.astype(np.int16)
    return np.tile(w16, (8, 1))


def _blockdiag(m: np.ndarray, n: int) -> np.ndarray:
    k, j = m.shape
    out = np.zeros((k * n, j * n), m.dtype)
    for i in range(n):
        out[i * k:(i + 1) * k, i * j:(i + 1) * j] = m
    return out


def _segments(sorted_usv: np.ndarray):
    """Per 512-chunk list of (rel_lo, rel_hi, u) for the sorted usv runs."""
    bounds = np.searchsorted(sorted_usv, np.arange(NU + 1))
    segs = [[] for _ in range(NCHUNK)]
    for u in range(NU):
        lo, hi = int(bounds[u]), int(bounds[u + 1])
        if hi <= lo:
            continue
        k0, k1 = lo // CH, (hi - 1) // CH
        for k in range(k0, k1 + 1):
            a0, a1 = max(lo, CH * k), min(hi, CH * (k + 1))
            segs[k].append((a0 - CH * k, a1 - CH * k, u))
    return segs


def _build_graph(segs):
    import concourse.bass as bass
    import concourse.mybir as mybir
    from concourse import bacc
    from concourse.tile import TileContext

    f32 = mybir.dt.float32
    f16 = mybir.dt.float16
    i16 = mybir.dt.int16
    AF = mybir.ActivationFunctionType
    OP = mybir.AluOpType

    nc = bacc.Bacc()

    # ---- dram parameters -------------------------------------------------
    task_e = nc.declare_dram_parameter("task_e", [BPC, NT, H], f32, isOutput=False)
    usv_e = nc.declare_dram_parameter("usv_e", [BPC, NU, H], f32, isOutput=False)
    glob = nc.declare_dram_parameter("glob", [BPC, H], f32, isOutput=False)
    idx_t = nc.declare_dram_parameter("idx_t", [128, A // 16], i16, isOutput=False)
    w1bd16 = nc.declare_dram_parameter("w1bd16", [128, 128], f16, isOutput=False)
    w1ubd16 = nc.declare_dram_parameter("w1ubd16", [128, 128], f16, isOutput=False)
    w1gbd = nc.declare_dram_parameter("w1gbd", [128, 128], f32, isOutput=False)
    b1_2 = nc.declare_dram_parameter("b1_2", [128, 1], f32, isOutput=False)
    w2bd16 = nc.declare_dram_parameter("w2bd16", [128, 64], f16, isOutput=False)
    b2q = nc.declare_dram_parameter("b2q", [128, 1], f32, isOutput=False)
    w3bd416 = nc.declare_dram_parameter("w3bd416", [128, 4], f16, isOutput=False)
    wc1 = nc.declare_dram_parameter("wc1", [H, 64], f32, isOutput=False)
    bc1c = nc.declare_dram_parameter("bc1c", [64, 1], f32, isOutput=False)
    wc2 = nc.declare_dram_parameter("wc2", [64, 32], f32, isOutput=False)
    bc2q = nc.declare_dram_parameter("bc2q", [32, 1], f32, isOutput=False)
    wc3 = nc.declare_dram_parameter("wc3", [32, 1], f32, isOutput=False)
    bc3q = nc.declare_dram_parameter("bc3q", [1, 1], f32, isOutput=False)
    ident = nc.declare_dram_parameter("ident", [128, 128], f32, isOutput=False)

    tbl_dram = nc.dram_tensor("tbl_dram", [NT, 512], f16)
    probs_out = nc.declare_dram_parameter("probs_out", [BPC, A], f16, isOutput=True)
    sv_out = nc.declare_dram_parameter("sv_out", [1, BPC], f32, isOutput=True)

    with TileContext(nc) as tc:
        with tc.tile_pool(name="const", bufs=1) as cst, \
             tc.tile_pool(name="pair", bufs=2) as pr, \
             tc.tile_pool(name="tables", bufs=3) as tbl, \
             tc.tile_pool(name="gath", bufs=1) as gpool, \
             tc.tile_pool(name="big", bufs=2) as big, \
             tc.tile_pool(name="spans", bufs=2) as sp, \
             tc.tile_pool(name="l2", bufs=3) as l2, \
             tc.tile_pool(name="out", bufs=1) as outp, \
             tc.tile_pool(name="ps_a", bufs=2, space="PSUM") as psa, \
             tc.tile_pool(name="ps_tr", bufs=2, space="PSUM") as pstr, \
             tc.tile_pool(name="ps_h2", bufs=2, space="PSUM") as ps2, \
             tc.tile_pool(name="ps_s", bufs=2, space="PSUM") as pss:

            def load_const(ext, shape, dtype=f32):
                t = cst.tile(shape, dtype, tag=ext.name)
                nc.sync.dma_start(out=t[:], in_=ext[:])
                return t

            identt = load_const(ident, [128, 128])
            identt16 = cst.tile([128, 128], f16, tag="ident16")
            nc.vector.tensor_copy(identt16[:], identt[:])
            idxt_sb = load_const(idx_t, [128, A // 16], i16)
            w1bd16_sb = load_const(w1bd16, [128, 128], f16)
            w1ubd16_sb = load_const(w1ubd16, [128, 128], f16)
            w1gbd_sb = load_const(w1gbd, [128, 128])
            b1_2_sb = load_const(b1_2, [128, 1])
            w2bd16_sb = load_const(w2bd16, [128, 64], f16)
            b2q_sb = load_const(b2q, [128, 1])
            w3bd416_sb = load_const(w3bd416, [128, 4], f16)
            wc1_sb = load_const(wc1, [H, 64])
            bc1c_sb = load_const(bc1c, [64, 1])
            wc2_sb = load_const(wc2, [64, 32])
            bc2q_sb = load_const(bc2q, [32, 1])
            wc3_sb = load_const(wc3, [32, 1])
            bc3q_sb = load_const(bc3q, [1, 1])

            # ---- globals, g1b1, critic (tiny, f32) ------------------------
            g2_sb = cst.tile([BPC, 128], f32)
            nc.sync.dma_start(out=g2_sb[:, 0:H], in_=glob[:])
            nc.sync.dma_start(out=g2_sb[:, H:128], in_=glob[:])
            ps_gT = psa.tile([128, BPC], f32, tag="A")
            nc.tensor.transpose(out=ps_gT[:], in_=g2_sb[:], identity=identt[:BPC, :BPC])
            gT2_sb = cst.tile([128, BPC], f32)
            nc.scalar.activation(gT2_sb[:], ps_gT[:], AF.Identity, bias=0.0, scale=1.0)

            ps_g1 = psa.tile([128, BPC], f32, tag="A")
            nc.tensor.matmul(ps_g1[:], w1gbd_sb[:], gT2_sb[:], start=True, stop=True)
            g1b1_sb = cst.tile([128, BPC], f32)
            nc.scalar.activation(g1b1_sb[:], ps_g1[:], AF.Identity, bias=b1_2_sb[:],
                                 scale=1.0)

            # critic
            ps_h1c = psa.tile([64, BPC], f32, tag="A")
            nc.tensor.matmul(ps_h1c[:], wc1_sb[:], gT2_sb[0:H, :], start=True, stop=True)
            ec = cst.tile([64, BPC], f32, tag="ec")
            rc = cst.tile([64, BPC], f32, tag="rc")
            nc.scalar.activation(ec[:], ps_h1c[:], AF.Exp, bias=bc1c_sb[:], scale=1.0)
            nc.scalar.activation(rc[:], ps_h1c[:], AF.Relu, bias=bc1c_sb[:], scale=1.0)
            nc.vector.tensor_scalar(ec[:], ec[:], 1.0, None, OP.min)
            h1ce = cst.tile([64, BPC], f32, tag="h1ce")
            nc.vector.tensor_tensor(h1ce[:], ec[:], rc[:], OP.add)
            ps_h2c = psa.tile([32, BPC], f32, tag="A")
            nc.tensor.matmul(ps_h2c[:], wc2_sb[:], h1ce[:], start=True, stop=True)
            ec2 = cst.tile([32, BPC], f32, tag="ec2")
            rc2 = cst.tile([32, BPC], f32, tag="rc2")
            nc.scalar.activation(ec2[:], ps_h2c[:], AF.Exp, bias=bc2q_sb[:], scale=1.0)
            nc.scalar.activation(rc2[:], ps_h2c[:], AF.Relu, bias=bc2q_sb[:], scale=1.0)
            nc.vector.tensor_scalar(ec2[:], ec2[:], 1.0, None, OP.min)
            h2ce = cst.tile([32, BPC], f32, tag="h2ce")
            nc.vector.tensor_tensor(h2ce[:], ec2[:], rc2[:], OP.add)
            ps_sv = psa.tile([1, BPC], f32, tag="A")
            nc.tensor.matmul(ps_sv[:], wc3_sb[:], h2ce[:], start=True, stop=True)
            sv_sb = cst.tile([1, BPC], f32, tag="svsb")
            nc.scalar.activation(sv_sb[:], ps_sv[:], AF.Identity, bias=bc3q_sb[:],
                                 scale=1.0)
            nc.sync.dma_start(out=sv_out[:], in_=sv_sb[:])

            # ---- preamble: one table for all pairs + usvcols + gathers -----
            table = tbl.tile([128, 8, 4, 128], f16, tag="table", bufs=1)
            usvcols = {}
            if True:
                for p in range(NPAIR):
                    b0, b1i = 2 * p, 2 * p + 1
                    dmae = (nc.sync, nc.scalar, nc.sync, nc.scalar)[p]
                    taskc = pr.tile([128, 2, 8, H], f32, tag="taskc", bufs=2)
                    for i, b in enumerate((b0, b1i)):
                        dmae.dma_start(
                            out=taskc[:, i, :, :],
                            in_=task_e[b].rearrange("(p r) h -> p r h", p=128))
                    taskc16 = pr.tile([128, 8, H], f16, tag="taskc16")
                    nc.vector.tensor_copy(taskc16[:], taskc[:, 1, :, :])
                    for half in range(2):
                        ps_taskT = pstr.tile([128, 512], f32, tag="tr")
                        for c in range(4):
                            cc = half * 4 + c
                            nc.tensor.transpose(
                                out=ps_taskT[0:H, 128 * c:128 * (c + 1)],
                                in_=taskc[:, 0, cc, :], identity=identt[:])
                            nc.tensor.matmul(
                                ps_taskT[H:128, 128 * c:128 * (c + 1)],
                                taskc16[:, cc, :], identt16[:],
                                start=True, stop=True, tile_position=(0, H))
                        taskT16 = pr.tile([128, 512], f16, tag="taskT16", bufs=2)
                        if half == 0:
                            nc.vector.tensor_copy(taskT16[:], ps_taskT[:])
                        else:
                            nc.scalar.activation(taskT16[:], ps_taskT[:],
                                                 AF.Identity, bias=0.0, scale=1.0)
                        for c in range(4):
                            s = half * 4 + c
                            ps_t1 = psa.tile([128, 128], f32, tag="A")
                            nc.tensor.matmul(ps_t1[:],
                                             taskT16[:, 128 * c:128 * (c + 1)],
                                             w1bd16_sb[:], start=True, stop=True)
                            if c % 2 == 0:
                                nc.vector.tensor_copy(table[:, s, p, :], ps_t1[:])
                            else:
                                nc.scalar.activation(table[:, s, p, :], ps_t1[:],
                                                     AF.Identity, bias=0.0,
                                                     scale=1.0)

                    # usvcol [128, 32] f32: col u = U1_pair[:, u] + g1 + b1
                    usvc = pr.tile([NU, 2, H], f32, tag="usvc", bufs=2)
                    for i, b in enumerate((b0, b1i)):
                        dmae.dma_start(out=usvc[:, i, :], in_=usv_e[b])
                    ps_usvT = psa.tile([128, NU], f32, tag="A")
                    nc.tensor.transpose(
                        out=ps_usvT[:],
                        in_=usvc[:].rearrange("u i h -> u (i h)"),
                        identity=identt[:NU, :NU])
                    u_sb16 = pr.tile([128, NU], f16, tag="usvT16")
                    nc.vector.tensor_copy(u_sb16[:], ps_usvT[:])
                    ps_u1 = psa.tile([128, NU], f32, tag="A")
                    nc.tensor.matmul(ps_u1[:], w1ubd16_sb[:], u_sb16[:],
                                     start=True, stop=True)
                    bias1 = pr.tile([128, 1], f32, tag="bias1")
                    nc.scalar.activation(bias1[0:H, :], g1b1_sb[0:H, b0:b0 + 1],
                                         AF.Identity, bias=0.0, scale=1.0)
                    nc.scalar.activation(bias1[H:128, :],
                                         g1b1_sb[H:128, b1i:b1i + 1],
                                         AF.Identity, bias=0.0, scale=1.0)
                    usvcol = tbl.tile([128, NU], f32, tag=f"usvcol{p}",
                                      name=f"usvcol_{p}")
                    nc.scalar.activation(usvcol[:], ps_u1[:], AF.Identity,
                                         bias=bias1[:], scale=1.0)
                    usvcols[p] = usvcol

                nc.sync.dma_start(
                    out=tbl_dram[:].rearrange("(p s) e -> p s e", p=128),
                    in_=table[:].rearrange("p s q e -> p s (q e)"))
                gath = gpool.tile([128, 64, 512], f16, tag="gath")
                for q in range(8):
                    nc.gpsimd.dma_gather(
                        out_ap=gath[:, 8 * q:8 * (q + 1), :],
                        in_ap=tbl_dram[:],
                        idxs_ap=idxt_sb[:, 64 * q:64 * (q + 1)],
                        num_idxs=1024,
                        num_idxs_reg=1024,
                        elem_size=512,
                        transpose=False,
                        queue_num=0,
                    )

            # ---- score targets --------------------------------------------
            es_g = [outp.tile([4, A], f16, tag=f"es{g}", name=f"es_g{g}")
                    for g in range(2)]
            sums_g = [outp.tile([4, NCHUNK], f32, tag=f"sums{g}", name=f"sums_g{g}")
                      for g in range(2)]

            # ---- main pipeline (2 pairs per group) -------------------------
            for span_i in range(NSPAN):
                for grp in range(NPAIR // 2):
                    subs = (2 * grp, 2 * grp + 1)
                    cr = {}
                    for si in range(2):
                        s_pair = 2 * grp + si
                        usvcol = usvcols[s_pair]
                        hp = big.tile([128, SPAN], f16, tag=f"h1p{si}")
                        for c4 in range(CPS):
                            k = span_i * CPS + c4
                            ps_tr = pstr.tile([128, CH], f16, tag="tr")
                            for b in range(CH // 128):
                                blk = (CH // 128) * k + b
                                nc.tensor.transpose(
                                    out=ps_tr[:, 128 * b:128 * (b + 1)],
                                    in_=gath[:, blk,
                                             128 * s_pair:128 * (s_pair + 1)],
                                    identity=identt16[:])
                            for (a0, a1, u) in segs[k]:
                                nc.vector.tensor_scalar(
                                    hp[:, CH * c4 + a0:CH * c4 + a1],
                                    ps_tr[:, a0:a1],
                                    usvcol[:, u:u + 1], None, OP.add)
                        et0 = sp.tile([128, SPAN], f16, tag="e0")
                        nc.scalar.activation(et0[:], hp[:], AF.Exp, bias=0.0, scale=1.0)
                        et = sp.tile([128, SPAN], f16, tag=f"e1_{si}")
                        nc.vector.tensor_scalar(et[:], et0[:], 1.0, None, OP.min)
                        rt = sp.tile([128, SPAN], f16, tag=f"r1_{si}")
                        nc.vector.tensor_scalar(rt[:], hp[:], 0.0, None, OP.max)
                        cr[si] = (et, rt)
                    for c4 in range(CPS):
                        k = span_i * CPS + c4
                        off = CH * c4
                        ps_h2 = ps2.tile([128, CH], f32, tag="h2")
                        for si in range(2):
                            et, rt = cr[si]
                            nc.tensor.matmul(
                                ps_h2[64 * si:64 * si + 64, :],
                                w2bd16_sb[:], et[:, off:off + CH],
                                start=True, stop=False, tile_position=(0, 64 * si))
                            nc.tensor.matmul(
                                ps_h2[64 * si:64 * si + 64, :],
                                w2bd16_sb[:], rt[:, off:off + CH],
                                start=False, stop=True, tile_position=(0, 64 * si))
                        e2t0 = l2.tile([128, CH], f16, tag="e20")
                        nc.scalar.activation(e2t0[:], ps_h2[:], AF.Exp,
                                             bias=b2q_sb[:], scale=1.0)
                        e2t = l2.tile([128, CH], f16, tag="e2")
                        nc.vector.tensor_scalar(e2t[:], e2t0[:], 1.0, None, OP.min)
                        r2t = l2.tile([128, CH], f16, tag="r2")
                        nc.vector.tensor_scalar(r2t[:], ps_h2[:], b2q_sb[:], 0.0,
                                                OP.add, OP.max)
                        ps_sk = pss.tile([4, CH], f32, tag="s")
                        nc.tensor.matmul(ps_sk[:], w3bd416_sb[:], e2t[:],
                                         start=True, stop=False)
                        nc.tensor.matmul(ps_sk[:], w3bd416_sb[:], r2t[:],
                                         start=False, stop=True)
                        nc.scalar.activation(
                            es_g[grp][:, CH * k:CH * (k + 1)], ps_sk[:],
                            AF.Exp, bias=0.0, scale=1.0,
                            accum_out=sums_g[grp][:, k:k + 1])

            # ---- softmax normalisation ------------------------------------
            es8 = outp.tile([BPC, A], f16, tag="es8")
            sums8 = outp.tile([BPC, NCHUNK], f32, tag="sums8")
            for g in range(2):
                nc.sync.dma_start(out=es8[4 * g:4 * g + 4, :], in_=es_g[g][:])
                nc.sync.dma_start(out=sums8[4 * g:4 * g + 4, :], in_=sums_g[g][:])
            ssum = outp.tile([BPC, 1], f32, tag="ssum")
            nc.vector.tensor_reduce(ssum[:], sums8[:], mybir.AxisListType.X, OP.add)
            rsum = outp.tile([BPC, 1], f32, tag="rsum")
            nc.vector.reciprocal(rsum[:], ssum[:])
            nc.vector.tensor_scalar(es8[:], es8[:], rsum[:], None, OP.mult)
            nc.sync.dma_start(out=probs_out[:], in_=es8[:])

    nc.compile()
    return nc


def _prep_static(inputs):
    """Host-side marshalling of weights/indices (tiny, O(weights + A))."""
    f = lambda x: np.asarray(x, np.float32)
    W1, b1 = f(inputs["W1"]), f(inputs["b1"])
    W2, b2 = f(inputs["W2"]), f(inputs["b2"])
    W3 = f(inputs["W3"])
    Wc1, bc1 = f(inputs["Wc1"]), f(inputs["bc1"])
    Wc2, bc2 = f(inputs["Wc2"]), f(inputs["bc2"])
    Wc3, bc3 = f(inputs["Wc3"]), f(inputs["bc3"])
    W1_t, W1_u, W1_g = W1[0:H], W1[H:2 * H], W1[2 * H:3 * H]

    task_ids = np.asarray(inputs["task_ids"])
    usv_ids = np.asarray(inputs["usv_ids"])
    order = np.argsort(usv_ids, kind="stable")

    d = {
        "idx_t": _wrap_idx(task_ids[order]),
        "w1bd16": _blockdiag(W1_t, 2).astype(np.float16),
        "w1ubd16": _blockdiag(W1_u, 2).astype(np.float16),
        "w1gbd": _blockdiag(W1_g, 2),
        "b1_2": np.tile(b1, 2)[:, None],
        "w2bd16": _blockdiag(W2, 2).astype(np.float16),
        "b2q": np.tile(b2 - W2.sum(0), 4)[:, None],
        "w3bd416": _blockdiag(W3, 4).astype(np.float16),
        "wc1": Wc1,
        "bc1c": bc1[:, None],
        "wc2": Wc2,
        "bc2q": (bc2 - Wc2.sum(0))[:, None],
        "wc3": Wc3,
        "bc3q": (bc3 - Wc3.sum(0)).reshape(1, 1),
        "ident": np.eye(128, dtype=np.float32),
    }
    return {k: np.ascontiguousarray(v) for k, v in d.items()}, order


def kernel(**inputs):
    from concourse.bass_utils import run_bass_kernel_spmd

    task_ids = np.asarray(inputs["task_ids"])
    usv_ids = np.asarray(inputs["usv_ids"])
    key = (task_ids.tobytes(), usv_ids.tobytes())
    if _CACHE.get("key") != key:
        order = np.argsort(usv_ids, kind="stable")
        _CACHE["nc"] = _build_graph(_segments(usv_ids[order]))
        _CACHE["key"] = key
    nc = _CACHE["nc"]

    static, order = _prep_static(inputs)
    inv = np.empty(A, np.int64)
    inv[order] = np.arange(A)
    task = np.ascontiguousarray(np.asarray(inputs["task_embed"], np.float32))
    usv = np.ascontiguousarray(np.asarray(inputs["usv_embed"], np.float32))
    glob = np.ascontiguousarray(np.asarray(inputs["global_embed"], np.float32))

    in_maps = []
    for c in range(NCORES):
        sl = slice(c * BPC, (c + 1) * BPC)
        m = dict(static)
        m["task_e"] = task[sl]
        m["usv_e"] = usv[sl]
        m["glob"] = glob[sl]
        in_maps.append(m)

    res = None
    for attempt in range(3):
        try:
            res = run_bass_kernel_spmd(nc, in_maps, core_ids=list(range(NCORES)))
            break
        except Exception:
            if attempt == 2:
                raise
    outs = res.results
    probs = np.concatenate([outs[c]["probs_out"] for c in range(NCORES)], axis=0)
    probs = probs[:, inv]
    sv = np.concatenate([outs[c]["sv_out"][0] for c in range(NCORES)], axis=0)[:, None]
    return probs.astype(np.float32), sv.astype(np.float32)
